# revision 2
# baseline (speedup 1.0000x reference)
"""Binary-tree gated-expert MoE (root -> 2 mid -> 4 leaf experts) on 8 trn2 cores.

Strategy: expert-parallel dispatch by leaf index. Tokens are grouped on the
host by their 2-bit routing path (leaf = 2*bit0 + bit1); each of the 8
NeuronCores processes one contiguous chunk of one leaf's tokens (cores are
apportioned to leaves proportionally to token counts, 2 cores/leaf in the
balanced case). A core then runs 3 chained dense [C,2048]x[2048,2048] layers
(root W0, mid W1[bit0], leaf W2[leaf]) with relu+bias, entirely on-chip.

Device kernel keeps activations transposed ([D, tokens] feature-major) so each
layer's matmul output (PSUM [fout, tok]) is directly the next layer's rhs.
Matmuls run in fp16 (same TensorE rate as bf16, 8x finer mantissa) with fp32
PSUM accumulation; weights are streamed from HBM as pre-tiled [16, 128, 2048]
stripes and used as the stationary operand.

Schedule notes (from NTFF trace analysis): the kernel is TensorE-bound at
~1.02 cycles/column, so the only wins are at the edges. The first m-pair's
weight stripes stream as interleaved k-range slices so the pair-0 k-loop
(which trickles behind the x input DMA) never waits on stripe m=1; the output
is fp16 (harness tolerance 2e-2, fp16 adds ~2e-4) to halve output DMA; and
the last m-pair's epilogues are chunked column-wise across ACT/DVE and both
DMA rings so the post-matmul tail is short.
"""

import numpy as np
from contextlib import ExitStack

import concourse.bass as bass
from concourse import bacc, mybir, tile
from concourse.bass_utils import run_bass_kernel_spmd

D = 2048
PT = 128           # partition tile
KT = D // PT       # 16 contraction tiles per layer
MT = D // PT       # 16 output-feature tiles per layer
N_CORES = 8

F32 = mybir.dt.float32
F16 = mybir.dt.float16
NP_F16 = np.float16

# cache of compiled bass programs keyed by padded capacity C
_compiled = {}
# stash of the last run's results so a harness can inspect exec_time_ns
last_results = None


def _prep_weight(W):
    """[D, D] -> [MT, 128, D] fp16: stripe m holds W[:, m*128:(m+1)*128]
    rearranged so partition p = contraction row within k-chunk, and the free
    dim is (k, fout-col) — i.e. out[m, p, k*128 + c] = W[k*128 + p, m*128 + c].
    Each [128, 2048] stripe then DMAs contiguously into SBUF and its k-th
    [128, 128] column block is exactly the lhsT (stationary) matmul operand."""
    W4 = W.reshape(KT, PT, MT, PT)
    return np.ascontiguousarray(
        W4.transpose(2, 1, 0, 3).reshape(MT, PT, D).astype(NP_F16)
    )


def _prep_bias(b0, b1e, b2l):
    """three [D] biases -> [128, 3*MT] f32 where column li*MT + m holds
    bias[li][m*128 : (m+1)*128] along partitions."""
    cols = []
    for b in (b0, b1e, b2l):
        cols.append(b.reshape(MT, PT).T)  # [128, MT]
    return np.ascontiguousarray(np.concatenate(cols, axis=1).astype(np.float32))


def _tiling(maxg):
    """Pick (TN, NT, C): NT token tiles, first NT-1 of width TN (<= 512, one
    PSUM bank of fp32) and a final tile of C - (NT-1)*TN, with C = maxg
    exactly (no padding beyond the max per-core group size)."""
    maxg = max(maxg, 256)
    NT = -(-maxg // 512)
    TN = -(-maxg // NT)
    return TN, NT, maxg


def _build(C, TN, NT):
    """Build + compile the 3-layer SPMD program for per-core capacity C.

    Layer-1 matmuls must consume the 16 k-chunks of the input as they stream
    in, so the m loop runs in pairs (6 PSUM tiles live per pair, 8 banks
    total): each pair's k-loop trickles behind the input DMA instead of one
    m-tile waiting for the entire input. Weight stripes ride the scalar
    (qActDynamicHW) DMA ring so they never queue behind the big input
    transfers on the sync (qSPDynamicHW) ring."""
    widths = [TN] * (NT - 1) + [C - (NT - 1) * TN]
    starts = [i * TN for i in range(NT)]

    nc = bacc.Bacc(
        "TRN2",
        target_bir_lowering=False,
        debug=False,
        enable_asserts=False,
        num_devices=N_CORES,
    )
    xT = nc.dram_tensor("xT", [D, C], F16, kind="ExternalInput").ap()
    w0 = nc.dram_tensor("w0", [MT, PT, D], F16, kind="ExternalInput").ap()
    w1 = nc.dram_tensor("w1", [MT, PT, D], F16, kind="ExternalInput").ap()
    w2 = nc.dram_tensor("w2", [MT, PT, D], F16, kind="ExternalInput").ap()
    bias = nc.dram_tensor("bias", [PT, 3 * MT], F32, kind="ExternalInput").ap()
    yT = nc.dram_tensor("yT", [D, C], F16, kind="ExternalOutput").ap()

    with tile.TileContext(nc) as tc, ExitStack() as ctx:
        wpool = ctx.enter_context(tc.tile_pool(name="w", bufs=4))
        hpool = ctx.enter_context(tc.tile_pool(name="h", bufs=1))
        pspool = ctx.enter_context(tc.tile_pool(name="ps", bufs=8, space="PSUM"))
        opool = ctx.enter_context(tc.tile_pool(name="o", bufs=8))
        cpool = ctx.enter_context(tc.tile_pool(name="c", bufs=1))

        hA = hpool.tile([PT, KT, C], F16, tag="hA")
        hB = hpool.tile([PT, KT, C], F16, tag="hB")

        # All early DMAs round-robin across the shared SDMA engines at packet
        # granularity, so emission order ~= bandwidth share. The first matmul
        # needs stripe (w0, m=0) k=0 + x chunk 0; the pair-0 k-loop then
        # consumes (m0,k)+(m1,k) every ~0.9us, while x chunks land every
        # ~1.9us. Stream the two stripes as interleaved k-range slices so
        # stripe m=1's early k-slices don't queue behind all of stripe m=0
        # (a full stripe is ~3.8us of queue time; a late m1 k=1 slice stalls
        # the PE at the start of the k-loop).
        wts0 = []
        for m in (0, 1):
            wt = wpool.tile([PT, D], F16, tag="wt", name=f"wt0_{m}")
            nc.scalar.dma_start(wt[:, 0:PT], w0[m, :, 0:PT])
            wts0.append(wt)
        nc.sync.dma_start(hA[:, 0, 0:TN], xT[0:PT, 0:TN])
        for ks in range(1, KT, 4):  # k-groups 1-4, 5-8, 9-12, 13-15
            ke = min(ks + 4, KT)
            for m in (0, 1):
                nc.scalar.dma_start(
                    wts0[m][:, ks * PT : ke * PT], w0[m, :, ks * PT : ke * PT]
                )
        if TN < C:
            nc.sync.dma_start(hA[:, 0, TN:C], xT[0:PT, TN:C])
        for k in range(1, KT):
            nc.sync.dma_start(hA[:, k, :], xT[k * PT : (k + 1) * PT, :])
        bias_sb = cpool.tile([PT, 3 * MT], F32)
        nc.scalar.dma_start(bias_sb[:], bias[:])

        def relu_bias(out_ap, ps_ap, b_ap, on_dve):
            if on_dve:
                nc.vector.tensor_scalar(
                    out_ap, ps_ap, b_ap, 0.0,
                    mybir.AluOpType.add, mybir.AluOpType.max,
                )
            else:
                nc.scalar.activation(
                    out_ap, ps_ap,
                    mybir.ActivationFunctionType.Relu, bias=b_ap,
                )

        layers = [(w0, 0, hA, hB), (w1, 1, hB, hA), (w2, 2, hA, None)]
        for w_dram, li, h_in, h_out in layers:
            for mp in range(MT // 2):
                ms = (2 * mp, 2 * mp + 1)
                if li == 0 and mp == 0:
                    wts = wts0
                else:
                    wts = []
                    halves = 2 if (li == 0 and mp == 1) else 1
                    for m in ms:
                        wt = wpool.tile([PT, D], F16, tag="wt", name=f"wt{li}_{m}")
                        wts.append(wt)
                    # pair 1 of layer 0 still races the pair-0 k-loop;
                    # interleave its two stripes as halves.
                    for hv in range(halves):
                        lo = hv * (D // halves)
                        hi = lo + D // halves
                        for mi, m in enumerate(ms):
                            nc.scalar.dma_start(
                                wts[mi][:, lo:hi], w_dram[m, :, lo:hi]
                            )
                pss = {
                    (m, n): pspool.tile([PT, TN], F32, tag="ps", name=f"ps{li}_{m}_{n}")
                    for m in ms
                    for n in range(NT)
                }

                def epilogue(mi, m, n):
                    n0, w = starts[n], widths[n]
                    b_ap = bias_sb[:, li * MT + m : li * MT + m + 1]
                    # alternate ACT/DVE so epilogues drain on two engines
                    on_dve = (n + mi) % 2 == 1
                    if h_out is not None:
                        relu_bias(
                            h_out[:, m, n0 : n0 + w], pss[(m, n)][:, :w],
                            b_ap, on_dve,
                        )
                    else:
                        # final layer: emit fp16 output tiles; the last
                        # m-pair's tiles are chunked column-wise so the tail
                        # after the final matmul (epilogue + out-DMA of the
                        # last chunk) is short.
                        is_last_pair = m >= MT - 2
                        is_last_tile = m == MT - 1 and n == NT - 1
                        nchunk = 4 if is_last_tile else (2 if is_last_pair else 1)
                        ot = opool.tile([PT, w], F16, tag="ot", name=f"ot{m}_{n}")
                        bounds = [w * i // nchunk for i in range(nchunk + 1)]
                        for ci in range(nchunk):
                            c0, c1 = bounds[ci], bounds[ci + 1]
                            relu_bias(
                                ot[:, c0:c1], pss[(m, n)][:, c0:c1],
                                b_ap, (on_dve + ci) % 2 == 1,
                            )
                            dma_eng = nc.sync if (on_dve + ci) % 2 == 1 else nc.scalar
                            dma_eng.dma_start(
                                yT[m * PT : (m + 1) * PT, n0 + c0 : n0 + c1],
                                ot[:, c0:c1],
                            )

                if li == 0:
                    # k-outer: consume the streaming input chunks as they land
                    for k in range(KT):
                        for mi, m in enumerate(ms):
                            for n in range(NT):
                                n0, w = starts[n], widths[n]
                                nc.tensor.matmul(
                                    pss[(m, n)][:, :w],
                                    wts[mi][:, k * PT : (k + 1) * PT],
                                    h_in[:, k, n0 : n0 + w],
                                    start=(k == 0),
                                    stop=(k == KT - 1),
                                    skip_group_check=True,
                                )
                    for mi, m in enumerate(ms):
                        for n in range(NT):
                            epilogue(mi, m, n)
                else:
                    # inputs resident: k-inner per tile, so each tile's
                    # epilogue (and final-layer out-DMA) fires as soon as its
                    # accumulation completes — the kernel tail drains one
                    # tile, not six
                    for mi, m in enumerate(ms):
                        for n in range(NT):
                            n0, w = starts[n], widths[n]
                            for k in range(KT):
                                nc.tensor.matmul(
                                    pss[(m, n)][:, :w],
                                    wts[mi][:, k * PT : (k + 1) * PT],
                                    h_in[:, k, n0 : n0 + w],
                                    start=(k == 0),
                                    stop=(k == KT - 1),
                                )
                            epilogue(mi, m, n)
    nc.compile()
    return nc


def _apportion_cores(counts):
    """Assign 8 cores to 4 leaves ~proportionally to token counts.
    Returns list of core counts per leaf (sums to N_CORES; 0 only for empty
    leaves). Greedy: repeatedly hand a core to the leaf with max load/core."""
    alive = [l for l in range(4) if counts[l] > 0]
    n = {l: 1 for l in alive}
    for _ in range(N_CORES - len(alive)):
        l = max(alive, key=lambda l: counts[l] / n[l])
        n[l] += 1
    return [n.get(l, 0) for l in range(4)]


def kernel(x, W0, b0, W1, b1, W2, b2, path_mask):
    global last_results
    x = np.asarray(x, dtype=np.float32)
    path_mask = np.asarray(path_mask)
    W0, b0, W1, b1, W2, b2 = (
        np.asarray(a, dtype=np.float32) for a in (W0, b0, W1, b1, W2, b2)
    )
    B = x.shape[0]

    bit0 = path_mask[:, 0].astype(np.int64)
    bit1 = path_mask[:, 1].astype(np.int64)
    leaf = 2 * bit0 + bit1
    order = np.argsort(leaf, kind="stable")
    counts = np.bincount(leaf, minlength=4)

    per_leaf = _apportion_cores(counts)
    # contiguous chunks of the leaf-sorted order per core
    groups = []      # list of (leaf, index-array) per core
    start = 0
    for l in range(4):
        cnt = int(counts[l])
        tok = order[start : start + cnt]
        start += cnt
        nl = per_leaf[l]
        if nl == 0:
            continue
        bounds = [round(i * cnt / nl) for i in range(nl + 1)]
        for i in range(nl):
            groups.append((l, tok[bounds[i] : bounds[i + 1]]))
    while len(groups) < N_CORES:  # only if some leaf was empty and slots remain
        groups.append((0, np.zeros(0, dtype=np.int64)))

    maxg = max(len(g[1]) for g in groups)
    TN, NT, C = _tiling(maxg)

    if C not in _compiled:
        _compiled[C] = _build(C, TN, NT)
    nc = _compiled[C]

    w_prepped = {}  # cache per (matrix id)
    def wp(tag, W):
        if tag not in w_prepped:
            w_prepped[tag] = _prep_weight(W)
        return w_prepped[tag]

    xb = x.astype(NP_F16)
    in_maps = []
    for l, tok in groups:
        xTg = np.zeros((D, C), dtype=NP_F16)
        if len(tok):
            xTg[:, : len(tok)] = xb[tok].T
        in_maps.append(
            {
                "xT": xTg,
                "w0": wp("w0", W0),
                "w1": wp(("w1", l // 2), W1[l // 2]),
                "w2": wp(("w2", l), W2[l]),
                "bias": _prep_bias(b0, b1[l // 2], b2[l]),
            }
        )

    last_results = run_bass_kernel_spmd(nc, in_maps, core_ids=list(range(N_CORES)))

    y = np.empty((B, D), dtype=np.float32)
    for (l, tok), res in zip(groups, last_results.results):
        if len(tok):
            y[tok] = res["yT"][:, : len(tok)].T.astype(np.float32)
    return y


# revision 5
# speedup vs baseline: 1.0018x; 1.0018x over previous
"""Binary-tree gated-expert MoE (root -> 2 mid -> 4 leaf experts) on 8 trn2 cores.

Strategy: expert-parallel dispatch by leaf index. Tokens are grouped on the
host by their 2-bit routing path (leaf = 2*bit0 + bit1); each of the 8
NeuronCores processes one contiguous chunk of one leaf's tokens (cores are
apportioned to leaves proportionally to token counts, 2 cores/leaf in the
balanced case). A core then runs 3 chained dense [C,2048]x[2048,2048] layers
(root W0, mid W1[bit0], leaf W2[leaf]) with relu+bias, entirely on-chip.

Device kernel keeps activations transposed ([D, tokens] feature-major) so each
layer's matmul output (PSUM [fout, tok]) is directly the next layer's rhs.
Matmuls run in fp16 (same TensorE rate as bf16, 8x finer mantissa) with fp32
PSUM accumulation; weights are streamed from HBM as pre-tiled [16, 128, 2048]
stripes and used as the stationary operand.

Schedule notes (from NTFF trace analysis): the kernel is TensorE-bound at
~1.02 cycles/column, so the only wins are at the edges. The first m-pair's
weight stripes stream as interleaved k-range slices so the pair-0 k-loop
(which trickles behind the x input DMA) never waits on stripe m=1; the output
is fp16 (harness tolerance 2e-2, fp16 adds ~2e-4) to halve output DMA; and
the last m-pair's epilogues are chunked column-wise across ACT/DVE and both
DMA rings so the post-matmul tail is short.
"""

import numpy as np
from contextlib import ExitStack

import concourse.bass as bass
from concourse import bacc, mybir, tile
from concourse.bass_utils import run_bass_kernel_spmd

D = 2048
PT = 128           # partition tile
KT = D // PT       # 16 contraction tiles per layer
MT = D // PT       # 16 output-feature tiles per layer
N_CORES = 8

F32 = mybir.dt.float32
F16 = mybir.dt.float16
NP_F16 = np.float16

# cache of compiled bass programs keyed by padded capacity C
_compiled = {}
# stash of the last run's results so a harness can inspect exec_time_ns
last_results = None


def _prep_weight(W):
    """[D, D] -> [MT, 128, D] fp16: stripe m holds W[:, m*128:(m+1)*128]
    rearranged so partition p = contraction row within k-chunk, and the free
    dim is (k, fout-col) — i.e. out[m, p, k*128 + c] = W[k*128 + p, m*128 + c].
    Each [128, 2048] stripe then DMAs contiguously into SBUF and its k-th
    [128, 128] column block is exactly the lhsT (stationary) matmul operand."""
    W4 = W.reshape(KT, PT, MT, PT)
    return np.ascontiguousarray(
        W4.transpose(2, 1, 0, 3).reshape(MT, PT, D).astype(NP_F16)
    )


def _prep_bias(b0, b1e, b2l):
    """three [D] biases -> [128, 3*MT] f32 where column li*MT + m holds
    bias[li][m*128 : (m+1)*128] along partitions."""
    cols = []
    for b in (b0, b1e, b2l):
        cols.append(b.reshape(MT, PT).T)  # [128, MT]
    return np.ascontiguousarray(np.concatenate(cols, axis=1).astype(np.float32))


def _tiling(maxg):
    """Pick (TN, NT, C): NT token tiles, first NT-1 of width TN (<= 512, one
    PSUM bank of fp32) and a final tile of C - (NT-1)*TN, with C = maxg
    exactly (no padding beyond the max per-core group size)."""
    maxg = max(maxg, 256)
    NT = -(-maxg // 512)
    TN = -(-maxg // NT)
    return TN, NT, maxg


def _build(C, TN, NT):
    """Build + compile the 3-layer SPMD program for per-core capacity C.

    Layer-1 matmuls must consume the 16 k-chunks of the input as they stream
    in, so the m loop runs in pairs (6 PSUM tiles live per pair, 8 banks
    total): each pair's k-loop trickles behind the input DMA instead of one
    m-tile waiting for the entire input. Weight stripes ride the scalar
    (qActDynamicHW) DMA ring so they never queue behind the big input
    transfers on the sync (qSPDynamicHW) ring."""
    widths = [TN] * (NT - 1) + [C - (NT - 1) * TN]
    starts = [i * TN for i in range(NT)]

    nc = bacc.Bacc(
        "TRN2",
        target_bir_lowering=False,
        debug=False,
        enable_asserts=False,
        num_devices=N_CORES,
    )
    xT = nc.dram_tensor("xT", [D, C], F16, kind="ExternalInput").ap()
    w0 = nc.dram_tensor("w0", [MT, PT, D], F16, kind="ExternalInput").ap()
    w1 = nc.dram_tensor("w1", [MT, PT, D], F16, kind="ExternalInput").ap()
    w2 = nc.dram_tensor("w2", [MT, PT, D], F16, kind="ExternalInput").ap()
    bias = nc.dram_tensor("bias", [PT, 3 * MT], F32, kind="ExternalInput").ap()
    yT = nc.dram_tensor("yT", [D, C], F16, kind="ExternalOutput").ap()

    with tile.TileContext(nc) as tc, ExitStack() as ctx:
        wpool = ctx.enter_context(tc.tile_pool(name="w", bufs=4))
        hpool = ctx.enter_context(tc.tile_pool(name="h", bufs=1))
        pspool = ctx.enter_context(tc.tile_pool(name="ps", bufs=8, space="PSUM"))
        opool = ctx.enter_context(tc.tile_pool(name="o", bufs=8))
        cpool = ctx.enter_context(tc.tile_pool(name="c", bufs=1))

        hA = hpool.tile([PT, KT, C], F16, tag="hA")
        hB = hpool.tile([PT, KT, C], F16, tag="hB")

        # All early DMAs round-robin across the shared SDMA engines at packet
        # granularity, so emission order ~= bandwidth share. The first matmul
        # needs stripe (w0, m=0) k=0 + x chunk 0; the pair-0 k-loop then
        # consumes (m0,k)+(m1,k) every ~0.9us, while x chunks land every
        # ~1.9us. Stream the two stripes as interleaved k-range slices so
        # stripe m=1's early k-slices don't queue behind all of stripe m=0
        # (a full stripe is ~3.8us of queue time; a late m1 k=1 slice stalls
        # the PE at the start of the k-loop).
        wts0 = []
        for m in (0, 1):
            wt = wpool.tile([PT, D], F16, tag="wt", name=f"wt0_{m}")
            nc.scalar.dma_start(wt[:, 0:PT], w0[m, :, 0:PT])
            wts0.append(wt)
        nc.sync.dma_start(hA[:, 0, 0:TN], xT[0:PT, 0:TN])
        for ks in range(1, KT, 4):  # k-groups 1-4, 5-8, 9-12, 13-15
            ke = min(ks + 4, KT)
            for m in (0, 1):
                nc.scalar.dma_start(
                    wts0[m][:, ks * PT : ke * PT], w0[m, :, ks * PT : ke * PT]
                )
        if TN < C:
            nc.sync.dma_start(hA[:, 0, TN:C], xT[0:PT, TN:C])
        # The early phase is aggregate-DMA-bandwidth-bound: the layer-0 k-loop
        # consumes one x chunk per ~0.87us while two queues deliver one per
        # ~1.05us. Round-robin the remaining chunks over the sync and gpsimd
        # rings (scalar carries the weight stripes) to raise the early
        # aggregate toward the per-core HBM share; gpsimd is otherwise idle.
        x_engines = [nc.sync, nc.gpsimd]
        for k in range(1, KT):
            x_engines[(k - 1) % 2].dma_start(hA[:, k, :], xT[k * PT : (k + 1) * PT, :])
        bias_sb = cpool.tile([PT, 3 * MT], F32)
        nc.scalar.dma_start(bias_sb[:], bias[:])

        def relu_bias(out_ap, ps_ap, b_ap, on_dve):
            if on_dve:
                nc.vector.tensor_scalar(
                    out_ap, ps_ap, b_ap, 0.0,
                    mybir.AluOpType.add, mybir.AluOpType.max,
                )
            else:
                nc.scalar.activation(
                    out_ap, ps_ap,
                    mybir.ActivationFunctionType.Relu, bias=b_ap,
                )

        layers = [(w0, 0, hA, hB), (w1, 1, hB, hA), (w2, 2, hA, None)]
        for w_dram, li, h_in, h_out in layers:
            for mp in range(MT // 2):
                ms = (2 * mp, 2 * mp + 1)
                if li == 0 and mp == 0:
                    wts = wts0
                else:
                    wts = []
                    halves = 2 if (li == 0 and mp == 1) else 1
                    for m in ms:
                        wt = wpool.tile([PT, D], F16, tag="wt", name=f"wt{li}_{m}")
                        wts.append(wt)
                    # pair 1 of layer 0 still races the pair-0 k-loop;
                    # interleave its two stripes as halves.
                    for hv in range(halves):
                        lo = hv * (D // halves)
                        hi = lo + D // halves
                        for mi, m in enumerate(ms):
                            nc.scalar.dma_start(
                                wts[mi][:, lo:hi], w_dram[m, :, lo:hi]
                            )
                pss = {
                    (m, n): pspool.tile([PT, TN], F32, tag="ps", name=f"ps{li}_{m}_{n}")
                    for m in ms
                    for n in range(NT)
                }

                def epilogue(mi, m, n):
                    n0, w = starts[n], widths[n]
                    b_ap = bias_sb[:, li * MT + m : li * MT + m + 1]
                    # alternate ACT/DVE so epilogues drain on two engines
                    on_dve = (n + mi) % 2 == 1
                    if h_out is not None:
                        relu_bias(
                            h_out[:, m, n0 : n0 + w], pss[(m, n)][:, :w],
                            b_ap, on_dve,
                        )
                    else:
                        # final layer: emit fp16 output tiles (harness
                        # tolerance is 2e-2; fp16 adds ~2e-4 and halves the
                        # output DMA, shortening the post-matmul tail)
                        ot = opool.tile([PT, w], F16, tag="ot", name=f"ot{m}_{n}")
                        relu_bias(ot[:], pss[(m, n)][:, :w], b_ap, on_dve)
                        dma_eng = nc.sync if on_dve else nc.scalar
                        dma_eng.dma_start(
                            yT[m * PT : (m + 1) * PT, n0 : n0 + w], ot[:]
                        )

                if li == 0:
                    # k-outer: consume the streaming input chunks as they land
                    for k in range(KT):
                        for mi, m in enumerate(ms):
                            for n in range(NT):
                                n0, w = starts[n], widths[n]
                                nc.tensor.matmul(
                                    pss[(m, n)][:, :w],
                                    wts[mi][:, k * PT : (k + 1) * PT],
                                    h_in[:, k, n0 : n0 + w],
                                    start=(k == 0),
                                    stop=(k == KT - 1),
                                    skip_group_check=True,
                                )
                    for mi, m in enumerate(ms):
                        for n in range(NT):
                            epilogue(mi, m, n)
                else:
                    # inputs resident: k-inner per tile, so each tile's
                    # epilogue (and final-layer out-DMA) fires as soon as its
                    # accumulation completes — the kernel tail drains one
                    # tile, not six
                    for mi, m in enumerate(ms):
                        for n in range(NT):
                            n0, w = starts[n], widths[n]
                            for k in range(KT):
                                nc.tensor.matmul(
                                    pss[(m, n)][:, :w],
                                    wts[mi][:, k * PT : (k + 1) * PT],
                                    h_in[:, k, n0 : n0 + w],
                                    start=(k == 0),
                                    stop=(k == KT - 1),
                                )
                            epilogue(mi, m, n)
    nc.compile()
    return nc


def _apportion_cores(counts):
    """Assign 8 cores to 4 leaves ~proportionally to token counts.
    Returns list of core counts per leaf (sums to N_CORES; 0 only for empty
    leaves). Greedy: repeatedly hand a core to the leaf with max load/core."""
    alive = [l for l in range(4) if counts[l] > 0]
    n = {l: 1 for l in alive}
    for _ in range(N_CORES - len(alive)):
        l = max(alive, key=lambda l: counts[l] / n[l])
        n[l] += 1
    return [n.get(l, 0) for l in range(4)]


def kernel(x, W0, b0, W1, b1, W2, b2, path_mask):
    global last_results
    x = np.asarray(x, dtype=np.float32)
    path_mask = np.asarray(path_mask)
    W0, b0, W1, b1, W2, b2 = (
        np.asarray(a, dtype=np.float32) for a in (W0, b0, W1, b1, W2, b2)
    )
    B = x.shape[0]

    bit0 = path_mask[:, 0].astype(np.int64)
    bit1 = path_mask[:, 1].astype(np.int64)
    leaf = 2 * bit0 + bit1
    order = np.argsort(leaf, kind="stable")
    counts = np.bincount(leaf, minlength=4)

    per_leaf = _apportion_cores(counts)
    # contiguous chunks of the leaf-sorted order per core
    groups = []      # list of (leaf, index-array) per core
    start = 0
    for l in range(4):
        cnt = int(counts[l])
        tok = order[start : start + cnt]
        start += cnt
        nl = per_leaf[l]
        if nl == 0:
            continue
        bounds = [round(i * cnt / nl) for i in range(nl + 1)]
        for i in range(nl):
            groups.append((l, tok[bounds[i] : bounds[i + 1]]))
    while len(groups) < N_CORES:  # only if some leaf was empty and slots remain
        groups.append((0, np.zeros(0, dtype=np.int64)))

    maxg = max(len(g[1]) for g in groups)
    TN, NT, C = _tiling(maxg)

    if C not in _compiled:
        _compiled[C] = _build(C, TN, NT)
    nc = _compiled[C]

    w_prepped = {}  # cache per (matrix id)
    def wp(tag, W):
        if tag not in w_prepped:
            w_prepped[tag] = _prep_weight(W)
        return w_prepped[tag]

    xb = x.astype(NP_F16)
    in_maps = []
    for l, tok in groups:
        xTg = np.zeros((D, C), dtype=NP_F16)
        if len(tok):
            xTg[:, : len(tok)] = xb[tok].T
        in_maps.append(
            {
                "xT": xTg,
                "w0": wp("w0", W0),
                "w1": wp(("w1", l // 2), W1[l // 2]),
                "w2": wp(("w2", l), W2[l]),
                "bias": _prep_bias(b0, b1[l // 2], b2[l]),
            }
        )

    last_results = run_bass_kernel_spmd(nc, in_maps, core_ids=list(range(N_CORES)))

    y = np.empty((B, D), dtype=np.float32)
    for (l, tok), res in zip(groups, last_results.results):
        if len(tok):
            y[tok] = res["yT"][:, : len(tok)].T.astype(np.float32)
    return y


# revision 9
# speedup vs baseline: 1.0022x; 1.0004x over previous
"""Binary-tree gated-expert MoE (root -> 2 mid -> 4 leaf experts) on 8 trn2 cores.

Strategy: expert-parallel dispatch by leaf index. Tokens are grouped on the
host by their 2-bit routing path (leaf = 2*bit0 + bit1); each of the 8
NeuronCores processes one contiguous chunk of one leaf's tokens (cores are
apportioned to leaves proportionally to token counts, 2 cores/leaf in the
balanced case). A core then runs 3 chained dense [C,2048]x[2048,2048] layers
(root W0, mid W1[bit0], leaf W2[leaf]) with relu+bias, entirely on-chip.

Device kernel keeps activations transposed ([D, tokens] feature-major) so each
layer's matmul output (PSUM [fout, tok]) is directly the next layer's rhs.
Matmuls run in fp16 (same TensorE rate as bf16, 8x finer mantissa) with fp32
PSUM accumulation; weights are streamed from HBM as pre-tiled [16, 128, 2048]
stripes and used as the stationary operand.

Schedule notes (from NTFF trace analysis): the kernel is TensorE-bound at
~1.02 cycles/column, so the only wins are at the edges. The first m-pair's
weight stripes stream as interleaved k-range slices so the pair-0 k-loop
(which trickles behind the x input DMA) never waits on stripe m=1; the output
is fp16 (harness tolerance 2e-2, fp16 adds ~2e-4) to halve output DMA; and
the last m-pair's epilogues are chunked column-wise across ACT/DVE and both
DMA rings so the post-matmul tail is short.
"""

import numpy as np
from contextlib import ExitStack

import concourse.bass as bass
from concourse import bacc, mybir, tile
from concourse.bass_utils import run_bass_kernel_spmd

D = 2048
PT = 128           # partition tile
KT = D // PT       # 16 contraction tiles per layer
MT = D // PT       # 16 output-feature tiles per layer
N_CORES = 8

F32 = mybir.dt.float32
F16 = mybir.dt.float16
NP_F16 = np.float16

# cache of compiled bass programs keyed by padded capacity C
_compiled = {}
# stash of the last run's results so a harness can inspect exec_time_ns
last_results = None


def _prep_weight(W):
    """[D, D] -> [MT, 128, D] fp16: stripe m holds W[:, m*128:(m+1)*128]
    rearranged so partition p = contraction row within k-chunk, and the free
    dim is (k, fout-col) — i.e. out[m, p, k*128 + c] = W[k*128 + p, m*128 + c].
    Each [128, 2048] stripe then DMAs contiguously into SBUF and its k-th
    [128, 128] column block is exactly the lhsT (stationary) matmul operand."""
    W4 = W.reshape(KT, PT, MT, PT)
    return np.ascontiguousarray(
        W4.transpose(2, 1, 0, 3).reshape(MT, PT, D).astype(NP_F16)
    )


def _prep_bias(b0, b1e, b2l):
    """three [D] biases -> [128, 3*MT] f32 where column li*MT + m holds
    bias[li][m*128 : (m+1)*128] along partitions."""
    cols = []
    for b in (b0, b1e, b2l):
        cols.append(b.reshape(MT, PT).T)  # [128, MT]
    return np.ascontiguousarray(np.concatenate(cols, axis=1).astype(np.float32))


def _tiling(maxg):
    """Pick (TN, NT, C): NT token tiles, first NT-1 of width TN (<= 512, one
    PSUM bank of fp32) and a final tile of C - (NT-1)*TN, with C = maxg
    exactly (no padding beyond the max per-core group size)."""
    maxg = max(maxg, 256)
    NT = -(-maxg // 512)
    TN = -(-maxg // NT)
    return TN, NT, maxg


def _build(C, TN, NT):
    """Build + compile the 3-layer SPMD program for per-core capacity C.

    Layer-1 matmuls must consume the 16 k-chunks of the input as they stream
    in, so the m loop runs in pairs (6 PSUM tiles live per pair, 8 banks
    total): each pair's k-loop trickles behind the input DMA instead of one
    m-tile waiting for the entire input. Weight stripes ride the scalar
    (qActDynamicHW) DMA ring so they never queue behind the big input
    transfers on the sync (qSPDynamicHW) ring."""
    widths = [TN] * (NT - 1) + [C - (NT - 1) * TN]
    starts = [i * TN for i in range(NT)]

    nc = bacc.Bacc(
        "TRN2",
        target_bir_lowering=False,
        debug=False,
        enable_asserts=False,
        num_devices=N_CORES,
    )
    xT = nc.dram_tensor("xT", [D, C], F16, kind="ExternalInput").ap()
    w0 = nc.dram_tensor("w0", [MT, PT, D], F16, kind="ExternalInput").ap()
    w1 = nc.dram_tensor("w1", [MT, PT, D], F16, kind="ExternalInput").ap()
    w2 = nc.dram_tensor("w2", [MT, PT, D], F16, kind="ExternalInput").ap()
    bias = nc.dram_tensor("bias", [PT, 3 * MT], F32, kind="ExternalInput").ap()
    yT = nc.dram_tensor("yT", [D, C], F16, kind="ExternalOutput").ap()

    with tile.TileContext(nc) as tc, ExitStack() as ctx:
        wpool = ctx.enter_context(tc.tile_pool(name="w", bufs=4))
        hpool = ctx.enter_context(tc.tile_pool(name="h", bufs=1))
        pspool = ctx.enter_context(tc.tile_pool(name="ps", bufs=8, space="PSUM"))
        opool = ctx.enter_context(tc.tile_pool(name="o", bufs=8))
        cpool = ctx.enter_context(tc.tile_pool(name="c", bufs=1))

        hA = hpool.tile([PT, KT, C], F16, tag="hA")
        hB = hpool.tile([PT, KT, C], F16, tag="hB")

        # PE p-state warmup: the PE runs at ~1.2GHz until it has executed
        # ~3us continuously, and the first real matmul can't start until the
        # first x chunk + w0 k0-slices land (~9.8us; the preamble ends ~7us).
        # Fill the wait with dummy matmuls on a zeroed SBUF tile so the ramp
        # completes before real data arrives and every real matmul runs at
        # full clock. The memset is gpsimd's first instruction so the dummies
        # aren't delayed behind its x-chunk DMA issues.
        warm = cpool.tile([PT, PT], F16)
        nc.gpsimd.memset(warm[:], 0.0)
        warm_ps = pspool.tile([PT, TN], F32, tag="ps", name="warm_ps")
        for r in range(28):
            nc.tensor.matmul(
                warm_ps[:, (r % 2) * PT : (r % 2 + 1) * PT], warm[:], warm[:],
                start=True, stop=True, skip_group_check=True,
            )

        # All early DMAs round-robin across the shared SDMA engines at packet
        # granularity, so emission order ~= bandwidth share. The first matmul
        # needs stripe (w0, m=0) k=0 + x chunk 0; the pair-0 k-loop then
        # consumes (m0,k)+(m1,k) every ~0.9us, while x chunks land every
        # ~1.9us. Stream the two stripes as interleaved k-range slices so
        # stripe m=1's early k-slices don't queue behind all of stripe m=0
        # (a full stripe is ~3.8us of queue time; a late m1 k=1 slice stalls
        # the PE at the start of the k-loop).
        wts0 = []
        for m in (0, 1):
            wt = wpool.tile([PT, D], F16, tag="wt", name=f"wt0_{m}")
            nc.scalar.dma_start(wt[:, 0:PT], w0[m, :, 0:PT])
            wts0.append(wt)
        nc.sync.dma_start(hA[:, 0, 0:TN], xT[0:PT, 0:TN])
        for ks in range(1, KT, 4):  # k-groups 1-4, 5-8, 9-12, 13-15
            ke = min(ks + 4, KT)
            for m in (0, 1):
                nc.scalar.dma_start(
                    wts0[m][:, ks * PT : ke * PT], w0[m, :, ks * PT : ke * PT]
                )
        if TN < C:
            nc.sync.dma_start(hA[:, 0, TN:C], xT[0:PT, TN:C])
        # The early phase is aggregate-DMA-bandwidth-bound: the layer-0 k-loop
        # consumes one x chunk per ~0.87us while two queues deliver one per
        # ~1.05us. Round-robin the remaining chunks over the sync and gpsimd
        # rings (scalar carries the weight stripes) to raise the early
        # aggregate toward the per-core HBM share; gpsimd is otherwise idle.
        x_engines = [nc.sync, nc.gpsimd]
        for k in range(1, KT):
            x_engines[(k - 1) % 2].dma_start(hA[:, k, :], xT[k * PT : (k + 1) * PT, :])
        bias_sb = cpool.tile([PT, 3 * MT], F32)
        nc.scalar.dma_start(bias_sb[:], bias[:])

        def relu_bias(out_ap, ps_ap, b_ap, on_dve):
            if on_dve:
                nc.vector.tensor_scalar(
                    out_ap, ps_ap, b_ap, 0.0,
                    mybir.AluOpType.add, mybir.AluOpType.max,
                )
            else:
                nc.scalar.activation(
                    out_ap, ps_ap,
                    mybir.ActivationFunctionType.Relu, bias=b_ap,
                )

        layers = [(w0, 0, hA, hB), (w1, 1, hB, hA), (w2, 2, hA, None)]
        for w_dram, li, h_in, h_out in layers:
            for mp in range(MT // 2):
                ms = (2 * mp, 2 * mp + 1)
                if li == 0 and mp == 0:
                    wts = wts0
                else:
                    wts = []
                    halves = 2 if (li == 0 and mp == 1) else 1
                    for m in ms:
                        wt = wpool.tile([PT, D], F16, tag="wt", name=f"wt{li}_{m}")
                        wts.append(wt)
                    # pair 1 of layer 0 still races the pair-0 k-loop;
                    # interleave its two stripes as halves.
                    for hv in range(halves):
                        lo = hv * (D // halves)
                        hi = lo + D // halves
                        for mi, m in enumerate(ms):
                            nc.scalar.dma_start(
                                wts[mi][:, lo:hi], w_dram[m, :, lo:hi]
                            )
                pss = {
                    (m, n): pspool.tile([PT, TN], F32, tag="ps", name=f"ps{li}_{m}_{n}")
                    for m in ms
                    for n in range(NT)
                }

                def epilogue(mi, m, n):
                    n0, w = starts[n], widths[n]
                    b_ap = bias_sb[:, li * MT + m : li * MT + m + 1]
                    # alternate ACT/DVE so epilogues drain on two engines
                    on_dve = (n + mi) % 2 == 1
                    if h_out is not None:
                        relu_bias(
                            h_out[:, m, n0 : n0 + w], pss[(m, n)][:, :w],
                            b_ap, on_dve,
                        )
                    else:
                        # final layer: emit fp16 output tiles (harness
                        # tolerance is 2e-2; fp16 adds ~2e-4 and halves the
                        # output DMA, shortening the post-matmul tail)
                        ot = opool.tile([PT, w], F16, tag="ot", name=f"ot{m}_{n}")
                        if m == MT - 1 and n == NT - 1:
                            # very last tile: halve across ACT/DVE and both
                            # DMA rings so the post-matmul tail is one half-
                            # tile epilogue + one small DMA deep
                            h1 = w // 2
                            relu_bias(ot[:, :h1], pss[(m, n)][:, :h1], b_ap, False)
                            nc.scalar.dma_start(
                                yT[m * PT : (m + 1) * PT, n0 : n0 + h1], ot[:, :h1]
                            )
                            relu_bias(ot[:, h1:w], pss[(m, n)][:, h1:w], b_ap, True)
                            nc.sync.dma_start(
                                yT[m * PT : (m + 1) * PT, n0 + h1 : n0 + w],
                                ot[:, h1:w],
                            )
                        else:
                            relu_bias(ot[:], pss[(m, n)][:, :w], b_ap, on_dve)
                            dma_eng = nc.sync if on_dve else nc.scalar
                            dma_eng.dma_start(
                                yT[m * PT : (m + 1) * PT, n0 : n0 + w], ot[:]
                            )

                if li == 0:
                    # k-outer: consume the streaming input chunks as they land
                    for k in range(KT):
                        for mi, m in enumerate(ms):
                            for n in range(NT):
                                n0, w = starts[n], widths[n]
                                nc.tensor.matmul(
                                    pss[(m, n)][:, :w],
                                    wts[mi][:, k * PT : (k + 1) * PT],
                                    h_in[:, k, n0 : n0 + w],
                                    start=(k == 0),
                                    stop=(k == KT - 1),
                                    skip_group_check=True,
                                )
                    for mi, m in enumerate(ms):
                        for n in range(NT):
                            epilogue(mi, m, n)
                else:
                    # inputs resident: k-inner per tile, so each tile's
                    # epilogue (and final-layer out-DMA) fires as soon as its
                    # accumulation completes — the kernel tail drains one
                    # tile, not six
                    for mi, m in enumerate(ms):
                        for n in range(NT):
                            n0, w = starts[n], widths[n]
                            for k in range(KT):
                                nc.tensor.matmul(
                                    pss[(m, n)][:, :w],
                                    wts[mi][:, k * PT : (k + 1) * PT],
                                    h_in[:, k, n0 : n0 + w],
                                    start=(k == 0),
                                    stop=(k == KT - 1),
                                )
                            epilogue(mi, m, n)
    nc.compile()
    return nc


def _apportion_cores(counts):
    """Assign 8 cores to 4 leaves ~proportionally to token counts.
    Returns list of core counts per leaf (sums to N_CORES; 0 only for empty
    leaves). Greedy: repeatedly hand a core to the leaf with max load/core."""
    alive = [l for l in range(4) if counts[l] > 0]
    n = {l: 1 for l in alive}
    for _ in range(N_CORES - len(alive)):
        l = max(alive, key=lambda l: counts[l] / n[l])
        n[l] += 1
    return [n.get(l, 0) for l in range(4)]


def kernel(x, W0, b0, W1, b1, W2, b2, path_mask):
    global last_results
    x = np.asarray(x, dtype=np.float32)
    path_mask = np.asarray(path_mask)
    W0, b0, W1, b1, W2, b2 = (
        np.asarray(a, dtype=np.float32) for a in (W0, b0, W1, b1, W2, b2)
    )
    B = x.shape[0]

    bit0 = path_mask[:, 0].astype(np.int64)
    bit1 = path_mask[:, 1].astype(np.int64)
    leaf = 2 * bit0 + bit1
    order = np.argsort(leaf, kind="stable")
    counts = np.bincount(leaf, minlength=4)

    per_leaf = _apportion_cores(counts)
    # contiguous chunks of the leaf-sorted order per core
    groups = []      # list of (leaf, index-array) per core
    start = 0
    for l in range(4):
        cnt = int(counts[l])
        tok = order[start : start + cnt]
        start += cnt
        nl = per_leaf[l]
        if nl == 0:
            continue
        bounds = [round(i * cnt / nl) for i in range(nl + 1)]
        for i in range(nl):
            groups.append((l, tok[bounds[i] : bounds[i + 1]]))
    while len(groups) < N_CORES:  # only if some leaf was empty and slots remain
        groups.append((0, np.zeros(0, dtype=np.int64)))

    maxg = max(len(g[1]) for g in groups)
    TN, NT, C = _tiling(maxg)

    if C not in _compiled:
        _compiled[C] = _build(C, TN, NT)
    nc = _compiled[C]

    w_prepped = {}  # cache per (matrix id)
    def wp(tag, W):
        if tag not in w_prepped:
            w_prepped[tag] = _prep_weight(W)
        return w_prepped[tag]

    xb = x.astype(NP_F16)
    in_maps = []
    for l, tok in groups:
        xTg = np.zeros((D, C), dtype=NP_F16)
        if len(tok):
            xTg[:, : len(tok)] = xb[tok].T
        in_maps.append(
            {
                "xT": xTg,
                "w0": wp("w0", W0),
                "w1": wp(("w1", l // 2), W1[l // 2]),
                "w2": wp(("w2", l), W2[l]),
                "bias": _prep_bias(b0, b1[l // 2], b2[l]),
            }
        )

    last_results = run_bass_kernel_spmd(nc, in_maps, core_ids=list(range(N_CORES)))

    y = np.empty((B, D), dtype=np.float32)
    for (l, tok), res in zip(groups, last_results.results):
        if len(tok):
            y[tok] = res["yT"][:, : len(tok)].T.astype(np.float32)
    return y
